# revision 23
# baseline (speedup 1.0000x reference)
"""Causal self-attention Trainium2 kernel (8-core SPMD).

Problem: x[2,2048,1024], causal mask, Wqkv[3072,1024], Wo[1024,1024], fp32.
  qkv = x @ Wqkv.T ; per-head causal softmax attention ; out = attn @ Wo.T

Sharding (data + tensor parallel, per the head dimension):
  core c -> batch b = c // 4, heads {4g..4g+3} with g = c % 4.
  Each core computes Q,K,V for its 4 heads (512 qk cols + 256 v cols of the
  projection), runs causal attention for those heads, and multiplies by the
  matching 256 columns of Wo, producing a partial [2048, 1024] output (fp16).
  Host sums the 8 partials (4 per batch) in fp32.

Kernel structure (per core):
  - bf16 matmul operands (PE 1 cyc/row), fp32 PSUM accumulation.
  - Inputs are staged in DRAM in SBUF-matching tile-major order and DMA'd
    in consumption order: wqkT[ko0] + xT[qc0,ko0] first so the first
    projection matmul can issue ~3us in, then the rest of q-chunk 0's
    operands, then later q-chunks. Projection chunks (ko-outer) interleave
    with attention chunks.
  - Scores are computed TRANSPOSED (scoresT[k, q], head pairs packed in the
    PE via partition-base row tiling) so AV needs no transposes. Score
    blocks go into 2-bank PSUM tiles (two k-blocks per tile) so one
    ACTIVATE exps 1024 columns, halving ACT instruction overhead.
  - Causality: strictly-upper blocks skipped; diagonal straddlers compute
    only the valid columns; the 128x128 diagonal sub-block is exp'd
    unmasked then multiplied by a binary mask tile (from the mask input).
  - V carries a ones column (65 cols/head): AV's partition 64 accumulates
    the softmax denominator for free. Normalization = fast-approx
    reciprocal (sums >= 1), broadcast over partitions via gpsimd, one DVE
    multiply.
  - Output partials are written fp16 (verified: no accuracy impact at the
    harness tolerance), halving output DMA bytes.
"""

import os

import numpy as np

S = 2048
D = 1024
DH = 64
B = 2
NCORES = 8
HPC = 4  # heads per core
QKC = 2 * HPC * DH  # 512 q+k projection columns per core
VC = HPC * DH  # 256 v columns per core
P = 128
KO = D // P  # 8 contraction tiles
NQ = S // 512  # 4 q-chunks of 512
NSC = S // P  # 16 s-chunks of 128

COMPUTE_DT = os.environ.get("ATTN_COMPUTE_DT", "bf16")  # bf16 | f32r

_cache = {}


def _np_compute_dt():
    if COMPUTE_DT == "bf16":
        import ml_dtypes

        return ml_dtypes.bfloat16
    return np.float32


def _build():
    import concourse.bacc as bacc
    import concourse.mybir as mybir
    import concourse.tile as tile

    F32 = mybir.dt.float32
    F16 = mybir.dt.float16
    CDT = mybir.dt.bfloat16 if COMPUTE_DT == "bf16" else mybir.dt.float32r
    EXP = mybir.ActivationFunctionType.Exp

    nc = bacc.Bacc()
    # x is staged twice: per-(qc,ko) contiguous 128KB tiles for q-chunk 0
    # (fine-grained arrival tracking at startup) and p-major whole-slab
    # layout for q-chunks 1..3 (single transfer each, arrives early enough).
    x0_d = nc.dram_tensor("x0", [KO, P, 512], CDT, kind="ExternalInput")
    xr_d = nc.dram_tensor("xr", [NQ - 1, P, KO, 512], CDT, kind="ExternalInput")
    wqkT_d = nc.dram_tensor("wqkT", [KO, P, QKC], CDT, kind="ExternalInput")
    wvT_d = nc.dram_tensor("wvT", [KO, P, VC], CDT, kind="ExternalInput")
    woT_d = nc.dram_tensor("woT", [P, 2, D], CDT, kind="ExternalInput")
    maskT_d = nc.dram_tensor("maskT", [P, P], CDT, kind="ExternalInput")
    out_d = nc.dram_tensor("out", [S, D], F16, kind="ExternalOutput")

    with tile.TileContext(nc) as tc:
        with (
            tc.tile_pool(name="persist", bufs=1) as persist,
            tc.tile_pool(name="sb_small", bufs=3) as sb_small,
            tc.tile_pool(name="sb_exp", bufs=18) as sb_exp,
            tc.tile_pool(name="sb_out", bufs=3) as sb_out,
            tc.tile_pool(name="pp_big", bufs=2, space="PSUM") as pp_big,
            tc.tile_pool(name="pp_av", bufs=2, space="PSUM") as pp_av,
            tc.tile_pool(name="pp_o", bufs=2, space="PSUM") as pp_o,
        ):
            xT_sb = persist.tile([P, KO, S], CDT, tag="xT")
            wqkT_sb = persist.tile([P, KO, QKC], CDT, tag="wqkT")
            wvT_sb = persist.tile([P, KO, VC], CDT, tag="wvT")
            woT_sb = persist.tile([P, 2, D], CDT, tag="woT")
            maskT_sb = persist.tile([P, P], CDT, tag="maskT")
            qkT_sb = persist.tile([P, 4, S], CDT, tag="qkT")
            v_sb = persist.tile([P, NSC, HPC, DH + 1], CDT, tag="v")
            attn_sb = persist.tile([P, 2, S], CDT, tag="attn")

            # --- input DMAs in consumption order, per-ko granularity for
            # everything q-chunk 0's projections touch so the PE can track
            # arrival tile by tile. Three issue queues: sync carries wqkT
            # (first m-tile split off so the first matmul can start on a
            # 32KB transfer), gpsimd carries x, scalar carries wvT (it is
            # otherwise idle until the first exp). xT for q-chunks 2/3 is
            # issued mid-attention (see below) to keep the early DMA window
            # uncongested.
            nc.sync.dma_start(wqkT_sb[:, 0, 0:P], wqkT_d[0][:, 0:P])
            nc.gpsimd.dma_start(xT_sb[:, 0, 0:512], x0_d[0])
            nc.sync.dma_start(wqkT_sb[:, 0, P:QKC], wqkT_d[0][:, P:QKC])
            for ko in range(1, KO):
                nc.sync.dma_start(wqkT_sb[:, ko, :], wqkT_d[ko])
                nc.gpsimd.dma_start(xT_sb[:, ko, 0:512], x0_d[ko])
            for ko in range(KO):
                nc.gpsimd.dma_start(wvT_sb[:, ko, :], wvT_d[ko])
            nc.sync.dma_start(maskT_sb[:], maskT_d[:])
            nc.gpsimd.dma_start(xT_sb[:, :, 512:1024], xr_d[0])

            ones_f32 = persist.tile([P, DH], F32, tag="ones_f32")
            nc.vector.memset(ones_f32[:], 1.0)
            ones_c = persist.tile([1, DH], CDT, tag="ones_c")
            nc.vector.memset(ones_c[:], 1.0)
            nc.vector.tensor_copy(
                out=v_sb[:, :, :, DH],
                in_=ones_f32[:, 0 : NSC * HPC].rearrange("p (a b) -> p a b", a=NSC),
            )

            def emit_outproj_sc(sc):
                for en in range(2):
                    ps_o = pp_o.tile([P, 512], F32, tag="o")
                    for ct in range(2):
                        nc.tensor.matmul(
                            ps_o[:],
                            attn_sb[:, ct, sc * P : (sc + 1) * P],
                            woT_sb[:, ct, en * 512 : (en + 1) * 512],
                            start=(ct == 0),
                            stop=(ct == 1),
                        )
                    o_sb = sb_out.tile([P, 512], F16, tag="osb")
                    nc.vector.tensor_copy(out=o_sb[:], in_=ps_o[:])
                    eng = nc.sync if (sc + en) % 2 == 0 else nc.gpsimd
                    eng.dma_start(
                        out_d[sc * P : (sc + 1) * P, en * 512 : (en + 1) * 512],
                        o_sb[:],
                    )

            for qc in range(NQ):
                # --- qk projection chunk nn = qc, ko-outer over two 2-bank
                # tiles (4 half-bank chains), so PE tracks DMA arrival ---
                pjA = pp_big.tile([P, 1024], F32, tag="big", name="pjA")
                pjB = pp_big.tile([P, 1024], F32, tag="big", name="pjB")
                for ko in range(KO):
                    for mm in range(4):
                        slot = pjA if mm < 2 else pjB
                        nc.tensor.matmul(
                            slot[:, (mm % 2) * 512 : (mm % 2 + 1) * 512],
                            wqkT_sb[:, ko, mm * P : (mm + 1) * P],
                            xT_sb[:, ko, qc * 512 : (qc + 1) * 512],
                            start=(ko == 0),
                            stop=(ko == KO - 1),
                            skip_group_check=True,
                        )
                nc.vector.tensor_copy(
                    out=qkT_sb[:, 0:2, qc * 512 : (qc + 1) * 512],
                    in_=pjA.rearrange("p (a b) -> p a b", a=2),
                )
                nc.vector.tensor_copy(
                    out=qkT_sb[:, 2:4, qc * 512 : (qc + 1) * 512],
                    in_=pjB.rearrange("p (a b) -> p a b", a=2),
                )

                # --- v projection for s-chunks 4qc..4qc+3 (4 bank chains) ---
                pvA = pp_big.tile([P, 1024], F32, tag="big", name="pvA")
                pvB = pp_big.tile([P, 1024], F32, tag="big", name="pvB")
                for ko in range(KO):
                    for j in range(4):
                        slot = pvA if j < 2 else pvB
                        sc = 4 * qc + j
                        nc.tensor.matmul(
                            slot[:, (j % 2) * 512 : (j % 2) * 512 + VC],
                            xT_sb[:, ko, sc * P : (sc + 1) * P],
                            wvT_sb[:, ko, :],
                            start=(ko == 0),
                            stop=(ko == KO - 1),
                            skip_group_check=True,
                        )
                for half, slot in ((0, pvA), (1, pvB)):
                    nc.vector.tensor_copy(
                        out=v_sb[:, 4 * qc + 2 * half : 4 * qc + 2 * half + 2, :, 0:DH],
                        in_=slot.rearrange("p (a h d) -> p a h d", a=2, h=8)[:, :, 0:HPC, :],
                    )

                # --- attention for q-chunk qc. The AV chain of head h is
                # emitted after head h+1's scores (cross-head software
                # pipeline): by the time the PE reaches AV(h), all of head
                # h's exps have long finished on ACT, so the PE never waits
                # on the activation engine at head boundaries. Same-kind
                # matmuls stay in long runs (pair-level interleaving
                # measurably thrashes the PE weight pipeline). ---
                nkb = 4 * qc + 4  # causal: k blocks 0 .. 4qc+3

                def emit_av_norm(avs, h):
                    hp = (h % 2) * DH
                    ps_av = pp_av.tile([DH + 1, 512], F32, tag="av")
                    for j, (exp2, lo, off, kb) in enumerate(avs):
                        nc.tensor.matmul(
                            ps_av[:, off:512],
                            v_sb[:, kb, h, :],
                            exp2[:, lo : (lo - off) + 512],
                            start=(j == 0),
                            stop=(j == len(avs) - 1),
                            skip_group_check=True,
                        )
                    # normalize: out = av * (1/sums) broadcast over partitions
                    sums_sb = sb_small.tile([1, 512], F32, tag="sums")
                    nc.vector.tensor_copy(out=sums_sb[:], in_=ps_av[DH : DH + 1, :])
                    recip_f = sb_small.tile([1, 512], F32, tag="recipf")
                    nc.vector.reciprocal_approx_fast(out=recip_f[:], in_=sums_sb[:])
                    if qc == NQ - 1 and h >= HPC - 2:
                        # last heads: the normalize latency is the kernel
                        # tail, so broadcast the reciprocal with a K=1
                        # ones-matmul on the PE (~0.25us) instead of the
                        # ~1us gpsimd broadcast.
                        recip_c = sb_small.tile([1, 512], CDT, tag="recipc")
                        nc.vector.tensor_copy(out=recip_c[:], in_=recip_f[:])
                        ps_bc = pp_av.tile([DH, 512], F32, tag="av", name="bc")
                        nc.tensor.matmul(
                            ps_bc[:],
                            ones_c[0:1, :],
                            recip_c[:],
                            start=True,
                            stop=True,
                            skip_group_check=True,
                        )
                        bc_sb = sb_small.tile([DH, 512], F32, tag="bc")
                        nc.vector.tensor_copy(out=bc_sb[:], in_=ps_bc[:])
                    else:
                        bc_sb = sb_small.tile([DH, 512], F32, tag="bc")
                        nc.gpsimd.partition_broadcast(bc_sb[:], recip_f[:])
                    nc.vector.tensor_mul(
                        out=attn_sb[hp : hp + DH, h // 2, qc * 512 : (qc + 1) * 512],
                        in0=ps_av[0:DH, :],
                        in1=bc_sb[:],
                    )

                pend = None  # (avs, h) of the previous head
                for h in range(HPC):
                    hp = (h % 2) * DH  # partition base within the m-tile
                    mq = h // 2  # Q m-tile; K m-tile = 2 + h//2
                    avs = []
                    for kb0 in range(0, nkb, 2):
                        ps2 = pp_big.tile([P, 1024], F32, tag="big", name="ps2")
                        exp2 = sb_exp.tile([P, 1024], CDT, tag="exp")
                        offs = []
                        for half in (0, 1):
                            kb = kb0 + half
                            m = kb - 4 * qc  # >= 0 on diagonal straddlers
                            off = max(0, m) * P
                            offs.append(off)
                            nc.tensor.matmul(
                                ps2[:, half * 512 + off : half * 512 + 512],
                                qkT_sb[hp : hp + DH, 2 + mq, kb * P : (kb + 1) * P],
                                qkT_sb[hp : hp + DH, mq, qc * 512 + off : (qc + 1) * 512],
                                start=True,
                                stop=True,
                                skip_group_check=True,
                            )
                        if offs[0] == 0 and offs[1] == 0:
                            # dense pair: one 1024-wide exp
                            nc.scalar.activation(exp2[:], ps2[:], EXP, scale=0.125)
                        else:
                            for half, off in enumerate(offs):
                                lo = half * 512 + off
                                nc.scalar.activation(
                                    exp2[:, lo : half * 512 + 512],
                                    ps2[:, lo : half * 512 + 512],
                                    EXP,
                                    scale=0.125,
                                )
                        for half, off in enumerate(offs):
                            kb = kb0 + half
                            if kb - 4 * qc >= 0:
                                lo = half * 512 + off
                                nc.vector.tensor_mul(
                                    out=exp2[:, lo : lo + P],
                                    in0=exp2[:, lo : lo + P],
                                    in1=maskT_sb[:],
                                )
                            avs.append((exp2, half * 512 + off, off, kb))
                    if pend is not None:
                        emit_av_norm(*pend)
                        # deferred output projection (previous q chunk), one
                        # s-chunk per head slot: dependency-free PE work
                        if qc > 0:
                            emit_outproj_sc(4 * (qc - 1) + pend[1])
                    pend = (avs, h)
                    # woT and xT for q-chunks 2/3 are issued here (scalar
                    # queue, between qc0's exps) so those 2.5MB stay out of
                    # the congested early DMA window; wo is first needed at
                    # ~30us, xqc2/xqc3 at ~50us+.
                    if qc == 0 and h == 0:
                        nc.scalar.dma_start(woT_sb[:], woT_d[:])
                    if qc == 0 and h in (1, 3):
                        qci = 2 if h == 1 else 3
                        nc.scalar.dma_start(
                            xT_sb[:, :, qci * 512 : (qci + 1) * 512],
                            xr_d[qci - 1],
                        )
                emit_av_norm(*pend)
                if qc > 0:
                    emit_outproj_sc(4 * (qc - 1) + pend[1])
            for si in range(4):
                emit_outproj_sc(4 * (NQ - 1) + si)

    nc.compile()
    return nc


def _get_nc():
    if "nc" not in _cache:
        _cache["nc"] = _build()
    return _cache["nc"]


def _shard(x, mask, Wqkv, Wo):
    cdt = _np_compute_dt()
    in_maps = []
    # binary mask for the transposed 128x128 diagonal block:
    # valid (mask==0) -> 1.0, masked (-inf/large-negative) -> 0.0
    maskT = np.ascontiguousarray((mask[0, 0, :P, :P].T >= 0).astype(cdt))
    for c in range(NCORES):
        b = c // 4
        g = c % 4
        heads = [4 * g + i for i in range(HPC)]
        q_rows = np.concatenate([np.arange(h * DH, (h + 1) * DH) for h in heads])
        k_rows = D + q_rows
        v_rows = 2 * D + q_rows
        qk_rows = np.concatenate([q_rows, k_rows])
        xT = x[b].T.astype(cdt)  # [D, S]
        x4 = xT.reshape(KO, P, NQ, 512)
        # q-chunk 0 per-(ko) contiguous tiles: [KO, P, 512]
        x0 = np.ascontiguousarray(x4[:, :, 0, :])
        # q-chunks 1..3 p-major slabs: [NQ-1, P, KO, 512]
        xr = np.ascontiguousarray(x4[:, :, 1:, :].transpose(2, 1, 0, 3))
        # [KO, P, QKC]: W[ko, p, m] = Wqkv[qk_rows[m], ko*P+p]
        wqkT = np.ascontiguousarray(Wqkv[qk_rows, :].T.astype(cdt).reshape(KO, P, QKC))
        wvT = np.ascontiguousarray(Wqkv[v_rows, :].T.astype(cdt).reshape(KO, P, VC))
        # [P, 2, D]: woT[p, ct, e] = Wo[e, q_rows[ct*P+p]]
        woT = np.ascontiguousarray(
            Wo[:, q_rows].T.astype(cdt).reshape(2, P, D).transpose(1, 0, 2)
        )
        in_maps.append(
            {
                "x0": x0,
                "xr": xr,
                "wqkT": wqkT,
                "wvT": wvT,
                "woT": woT,
                "maskT": maskT,
            }
        )
    return in_maps


def kernel(x, mask, Wqkv, Wo, _trace=False):
    from concourse.bass_utils import run_bass_kernel_spmd

    x = np.asarray(x, dtype=np.float32)
    mask = np.asarray(mask, dtype=np.float32)
    Wqkv = np.asarray(Wqkv, dtype=np.float32)
    Wo = np.asarray(Wo, dtype=np.float32)

    nc = _get_nc()
    in_maps = _shard(x, mask, Wqkv, Wo)
    res = run_bass_kernel_spmd(nc, in_maps, core_ids=list(range(NCORES)), trace=_trace)
    _cache["last_result"] = res

    out = np.zeros((B, S, D), dtype=np.float32)
    for c in range(NCORES):
        out[c // 4] += res.results[c]["out"].astype(np.float32)
    return out


# revision 27
# speedup vs baseline: 1.0350x; 1.0350x over previous
"""Causal self-attention Trainium2 kernel (8-core SPMD).

Problem: x[2,2048,1024], causal mask, Wqkv[3072,1024], Wo[1024,1024], fp32.
  qkv = x @ Wqkv.T ; per-head causal softmax attention ; out = attn @ Wo.T

Sharding (data + tensor parallel, per the head dimension):
  core c -> batch b = c // 4, heads {4g..4g+3} with g = c % 4.
  Each core computes Q,K,V for its 4 heads (512 qk cols + 256 v cols of the
  projection), runs causal attention for those heads, and multiplies by the
  matching 256 columns of Wo, producing a partial [2048, 1024] output (fp16).
  Host sums the 8 partials (4 per batch) in fp32.

Kernel structure (per core):
  - bf16 matmul operands (PE 1 cyc/row), fp32 PSUM accumulation.
  - Inputs are staged in DRAM in SBUF-matching tile-major order and DMA'd
    in consumption order: wqkT[ko0] + xT[qc0,ko0] first so the first
    projection matmul can issue ~3us in, then the rest of q-chunk 0's
    operands, then later q-chunks. Projection chunks (ko-outer) interleave
    with attention chunks.
  - Scores are computed TRANSPOSED (scoresT[k, q], head pairs packed in the
    PE via partition-base row tiling) so AV needs no transposes. Score
    blocks go into 2-bank PSUM tiles (two k-blocks per tile) so one
    ACTIVATE exps 1024 columns, halving ACT instruction overhead.
  - Causality: strictly-upper blocks skipped; diagonal straddlers compute
    only the valid columns; the 128x128 diagonal sub-block is exp'd
    unmasked then multiplied by a binary mask tile (from the mask input).
  - V carries a ones column (65 cols/head): AV's partition 64 accumulates
    the softmax denominator for free. Normalization = fast-approx
    reciprocal (sums >= 1), broadcast over partitions via gpsimd, one DVE
    multiply.
  - Output partials are written fp16 (verified: no accuracy impact at the
    harness tolerance), halving output DMA bytes.
"""

import os

import numpy as np

S = 2048
D = 1024
DH = 64
B = 2
NCORES = 8
HPC = 4  # heads per core
QKC = 2 * HPC * DH  # 512 q+k projection columns per core
VC = HPC * DH  # 256 v columns per core
P = 128
KO = D // P  # 8 contraction tiles
NQ = S // 512  # 4 q-chunks of 512
NSC = S // P  # 16 s-chunks of 128

COMPUTE_DT = os.environ.get("ATTN_COMPUTE_DT", "bf16")  # bf16 | f32r

_cache = {}


def _np_compute_dt():
    if COMPUTE_DT == "bf16":
        import ml_dtypes

        return ml_dtypes.bfloat16
    return np.float32


def _build():
    import concourse.bacc as bacc
    import concourse.mybir as mybir
    import concourse.tile as tile

    F32 = mybir.dt.float32
    F16 = mybir.dt.float16
    CDT = mybir.dt.bfloat16 if COMPUTE_DT == "bf16" else mybir.dt.float32r
    EXP = mybir.ActivationFunctionType.Exp

    nc = bacc.Bacc()
    # x is staged twice: per-(qc,ko) contiguous 128KB tiles for q-chunk 0
    # (fine-grained arrival tracking at startup) and p-major whole-slab
    # layout for q-chunks 1..3 (single transfer each, arrives early enough).
    x0_d = nc.dram_tensor("x0", [KO, P, 512], CDT, kind="ExternalInput")
    xr_d = nc.dram_tensor("xr", [NQ - 1, P, KO, 512], CDT, kind="ExternalInput")
    wqkT_d = nc.dram_tensor("wqkT", [KO, P, QKC], CDT, kind="ExternalInput")
    wvT_d = nc.dram_tensor("wvT", [KO, P, VC], CDT, kind="ExternalInput")
    woT_d = nc.dram_tensor("woT", [P, 2, D], CDT, kind="ExternalInput")
    maskT_d = nc.dram_tensor("maskT", [P, P], CDT, kind="ExternalInput")
    out_d = nc.dram_tensor("out", [S, D], F16, kind="ExternalOutput")

    with tile.TileContext(nc) as tc:
        with (
            tc.tile_pool(name="persist", bufs=1) as persist,
            tc.tile_pool(name="sb_small", bufs=3) as sb_small,
            tc.tile_pool(name="sb_exp", bufs=12) as sb_exp,
            tc.tile_pool(name="sb_out", bufs=3) as sb_out,
            tc.tile_pool(name="pp_big", bufs=2, space="PSUM") as pp_big,
            tc.tile_pool(name="pp_av", bufs=2, space="PSUM") as pp_av,
            tc.tile_pool(name="pp_o", bufs=2, space="PSUM") as pp_o,
        ):
            xT_sb = persist.tile([P, KO, S], CDT, tag="xT")
            wqkT_sb = persist.tile([P, KO, QKC], CDT, tag="wqkT")
            wvT_sb = persist.tile([P, KO, VC], CDT, tag="wvT")
            woT_sb = persist.tile([P, 2, D], CDT, tag="woT")
            maskT_sb = persist.tile([P, P], CDT, tag="maskT")
            qkT_sb = persist.tile([P, 4, S], CDT, tag="qkT")
            v_sb = persist.tile([P, NSC, HPC, DH + 1], CDT, tag="v")
            attn_sb = persist.tile([P, 2, S], CDT, tag="attn")

            # --- input DMAs in consumption order, per-ko granularity for
            # everything q-chunk 0's projections touch so the PE can track
            # arrival tile by tile. sync carries weights (first wqkT m-tile
            # split off so the first matmul can start on a 32KB transfer),
            # gpsimd carries x. xT for q-chunks 2/3 is emitted later on the
            # sync queue BEHIND output DMAs whose semaphores only clear
            # mid-kernel — engine queues run ahead of emission order, so
            # queue position behind a blocking wait is the only way to
            # actually defer those 2MB out of the congested startup window.
            nc.sync.dma_start(wqkT_sb[:, 0, 0:P], wqkT_d[0][:, 0:P])
            nc.gpsimd.dma_start(xT_sb[:, 0, 0:512], x0_d[0])
            nc.sync.dma_start(wqkT_sb[:, 0, P:QKC], wqkT_d[0][:, P:QKC])
            for ko in range(1, KO):
                nc.sync.dma_start(wqkT_sb[:, ko, :], wqkT_d[ko])
                nc.gpsimd.dma_start(xT_sb[:, ko, 0:512], x0_d[ko])
            for ko in range(KO):
                nc.sync.dma_start(wvT_sb[:, ko, :], wvT_d[ko])
            nc.sync.dma_start(maskT_sb[:], maskT_d[:])
            nc.gpsimd.dma_start(xT_sb[:, :, 512:1024], xr_d[0])
            nc.sync.dma_start(woT_sb[:], woT_d[:])

            ones_f32 = persist.tile([P, DH], F32, tag="ones_f32")
            nc.vector.memset(ones_f32[:], 1.0)
            nc.vector.tensor_copy(
                out=v_sb[:, :, :, DH],
                in_=ones_f32[:, 0 : NSC * HPC].rearrange("p (a b) -> p a b", a=NSC),
            )

            def emit_outproj_sc(sc):
                for en in range(2):
                    ps_o = pp_o.tile([P, 512], F32, tag="o")
                    for ct in range(2):
                        nc.tensor.matmul(
                            ps_o[:],
                            attn_sb[:, ct, sc * P : (sc + 1) * P],
                            woT_sb[:, ct, en * 512 : (en + 1) * 512],
                            start=(ct == 0),
                            stop=(ct == 1),
                        )
                    o_sb = sb_out.tile([P, 512], F16, tag="osb")
                    nc.vector.tensor_copy(out=o_sb[:], in_=ps_o[:])
                    eng = nc.sync if (sc + en) % 2 == 0 else nc.gpsimd
                    eng.dma_start(
                        out_d[sc * P : (sc + 1) * P, en * 512 : (en + 1) * 512],
                        o_sb[:],
                    )

            for qc in range(NQ):
                # --- qk projection chunk nn = qc, ko-outer over two 2-bank
                # tiles (4 half-bank chains), so PE tracks DMA arrival ---
                pjA = pp_big.tile([P, 1024], F32, tag="big", name="pjA")
                pjB = pp_big.tile([P, 1024], F32, tag="big", name="pjB")
                for ko in range(KO):
                    for mm in range(4):
                        slot = pjA if mm < 2 else pjB
                        nc.tensor.matmul(
                            slot[:, (mm % 2) * 512 : (mm % 2 + 1) * 512],
                            wqkT_sb[:, ko, mm * P : (mm + 1) * P],
                            xT_sb[:, ko, qc * 512 : (qc + 1) * 512],
                            start=(ko == 0),
                            stop=(ko == KO - 1),
                            skip_group_check=True,
                        )
                nc.vector.tensor_copy(
                    out=qkT_sb[:, 0:2, qc * 512 : (qc + 1) * 512],
                    in_=pjA.rearrange("p (a b) -> p a b", a=2),
                )
                nc.vector.tensor_copy(
                    out=qkT_sb[:, 2:4, qc * 512 : (qc + 1) * 512],
                    in_=pjB.rearrange("p (a b) -> p a b", a=2),
                )

                # --- v projection for s-chunks 4qc..4qc+3 (4 bank chains) ---
                pvA = pp_big.tile([P, 1024], F32, tag="big", name="pvA")
                pvB = pp_big.tile([P, 1024], F32, tag="big", name="pvB")
                for ko in range(KO):
                    for j in range(4):
                        slot = pvA if j < 2 else pvB
                        sc = 4 * qc + j
                        nc.tensor.matmul(
                            slot[:, (j % 2) * 512 : (j % 2) * 512 + VC],
                            xT_sb[:, ko, sc * P : (sc + 1) * P],
                            wvT_sb[:, ko, :],
                            start=(ko == 0),
                            stop=(ko == KO - 1),
                            skip_group_check=True,
                        )
                for half, slot in ((0, pvA), (1, pvB)):
                    nc.vector.tensor_copy(
                        out=v_sb[:, 4 * qc + 2 * half : 4 * qc + 2 * half + 2, :, 0:DH],
                        in_=slot.rearrange("p (a h d) -> p a h d", a=2, h=8)[:, :, 0:HPC, :],
                    )

                # --- attention for q-chunk qc ---
                nkb = 4 * qc + 4  # causal: k blocks 0 .. 4qc+3
                for h in range(HPC):
                    hp = (h % 2) * DH  # partition base within the m-tile
                    mq = h // 2  # Q m-tile; K m-tile = 2 + h//2
                    avs = []
                    for kb0 in range(0, nkb, 2):
                        ps2 = pp_big.tile([P, 1024], F32, tag="big", name="ps2")
                        exp2 = sb_exp.tile([P, 1024], CDT, tag="exp")
                        offs = []
                        for half in (0, 1):
                            kb = kb0 + half
                            m = kb - 4 * qc  # >= 0 on diagonal straddlers
                            off = max(0, m) * P
                            offs.append(off)
                            nc.tensor.matmul(
                                ps2[:, half * 512 + off : half * 512 + 512],
                                qkT_sb[hp : hp + DH, 2 + mq, kb * P : (kb + 1) * P],
                                qkT_sb[hp : hp + DH, mq, qc * 512 + off : (qc + 1) * 512],
                                start=True,
                                stop=True,
                                skip_group_check=True,
                            )
                        if offs[0] == 0 and offs[1] == 0:
                            # dense pair: one 1024-wide exp
                            nc.scalar.activation(exp2[:], ps2[:], EXP, scale=0.125)
                        else:
                            for half, off in enumerate(offs):
                                lo = half * 512 + off
                                nc.scalar.activation(
                                    exp2[:, lo : half * 512 + 512],
                                    ps2[:, lo : half * 512 + 512],
                                    EXP,
                                    scale=0.125,
                                )
                        for half, off in enumerate(offs):
                            kb = kb0 + half
                            if kb - 4 * qc >= 0:
                                lo = half * 512 + off
                                nc.vector.tensor_mul(
                                    out=exp2[:, lo : lo + P],
                                    in0=exp2[:, lo : lo + P],
                                    in1=maskT_sb[:],
                                )
                            avs.append((exp2, half * 512 + off, off, kb))
                    ps_av = pp_av.tile([DH + 1, 512], F32, tag="av")
                    for j, (exp2, lo, off, kb) in enumerate(avs):
                        nc.tensor.matmul(
                            ps_av[:, off:512],
                            v_sb[:, kb, h, :],
                            exp2[:, lo : (lo - off) + 512],
                            start=(j == 0),
                            stop=(j == len(avs) - 1),
                            skip_group_check=True,
                        )
                    # normalize: out = av * (1/sums) broadcast over partitions
                    sums_sb = sb_small.tile([1, 512], F32, tag="sums")
                    nc.vector.tensor_copy(out=sums_sb[:], in_=ps_av[DH : DH + 1, :])
                    recip_f = sb_small.tile([1, 512], F32, tag="recipf")
                    nc.vector.reciprocal_approx_fast(out=recip_f[:], in_=sums_sb[:])
                    bc_sb = sb_small.tile([DH, 512], F32, tag="bc")
                    nc.gpsimd.partition_broadcast(bc_sb[:], recip_f[:])
                    nc.vector.tensor_mul(
                        out=attn_sb[hp : hp + DH, h // 2, qc * 512 : (qc + 1) * 512],
                        in0=ps_av[0:DH, :],
                        in1=bc_sb[:],
                    )
                    # deferred output projection (previous q chunk), one
                    # s-chunk per head: dependency-free PE work between
                    # heads so the exp pipeline never starves the PE
                    if qc > 0:
                        emit_outproj_sc(4 * (qc - 1) + h)
                        # xT for q-chunks 2/3: emitted on sync BEHIND the
                        # outproj output DMA above, whose semaphore only
                        # clears mid-kernel — a genuine deferral of these
                        # 2MB past the congested startup window. Needed at
                        # ~55us / ~75us respectively.
                        if qc == 1 and h in (0, 3):
                            qci = 2 if h == 0 else 3
                            nc.sync.dma_start(
                                xT_sb[:, :, qci * 512 : (qci + 1) * 512],
                                xr_d[qci - 1],
                            )
            for si in range(4):
                emit_outproj_sc(4 * (NQ - 1) + si)

    nc.compile()
    return nc


def _get_nc():
    if "nc" not in _cache:
        _cache["nc"] = _build()
    return _cache["nc"]


def _shard(x, mask, Wqkv, Wo):
    cdt = _np_compute_dt()
    in_maps = []
    # binary mask for the transposed 128x128 diagonal block:
    # valid (mask==0) -> 1.0, masked (-inf/large-negative) -> 0.0
    maskT = np.ascontiguousarray((mask[0, 0, :P, :P].T >= 0).astype(cdt))
    for c in range(NCORES):
        b = c // 4
        g = c % 4
        heads = [4 * g + i for i in range(HPC)]
        q_rows = np.concatenate([np.arange(h * DH, (h + 1) * DH) for h in heads])
        k_rows = D + q_rows
        v_rows = 2 * D + q_rows
        qk_rows = np.concatenate([q_rows, k_rows])
        xT = x[b].T.astype(cdt)  # [D, S]
        x4 = xT.reshape(KO, P, NQ, 512)
        # q-chunk 0 per-(ko) contiguous tiles: [KO, P, 512]
        x0 = np.ascontiguousarray(x4[:, :, 0, :])
        # q-chunks 1..3 p-major slabs: [NQ-1, P, KO, 512]
        xr = np.ascontiguousarray(x4[:, :, 1:, :].transpose(2, 1, 0, 3))
        # [KO, P, QKC]: W[ko, p, m] = Wqkv[qk_rows[m], ko*P+p]
        wqkT = np.ascontiguousarray(Wqkv[qk_rows, :].T.astype(cdt).reshape(KO, P, QKC))
        wvT = np.ascontiguousarray(Wqkv[v_rows, :].T.astype(cdt).reshape(KO, P, VC))
        # [P, 2, D]: woT[p, ct, e] = Wo[e, q_rows[ct*P+p]]
        woT = np.ascontiguousarray(
            Wo[:, q_rows].T.astype(cdt).reshape(2, P, D).transpose(1, 0, 2)
        )
        in_maps.append(
            {
                "x0": x0,
                "xr": xr,
                "wqkT": wqkT,
                "wvT": wvT,
                "woT": woT,
                "maskT": maskT,
            }
        )
    return in_maps


def kernel(x, mask, Wqkv, Wo, _trace=False):
    from concourse.bass_utils import run_bass_kernel_spmd

    x = np.asarray(x, dtype=np.float32)
    mask = np.asarray(mask, dtype=np.float32)
    Wqkv = np.asarray(Wqkv, dtype=np.float32)
    Wo = np.asarray(Wo, dtype=np.float32)

    nc = _get_nc()
    in_maps = _shard(x, mask, Wqkv, Wo)
    res = run_bass_kernel_spmd(nc, in_maps, core_ids=list(range(NCORES)), trace=_trace)
    _cache["last_result"] = res

    out = np.zeros((B, S, D), dtype=np.float32)
    for c in range(NCORES):
        out[c // 4] += res.results[c]["out"].astype(np.float32)
    return out


# revision 28
# speedup vs baseline: 1.0413x; 1.0061x over previous
"""Causal self-attention Trainium2 kernel (8-core SPMD).

Problem: x[2,2048,1024], causal mask, Wqkv[3072,1024], Wo[1024,1024], fp32.
  qkv = x @ Wqkv.T ; per-head causal softmax attention ; out = attn @ Wo.T

Sharding (data + tensor parallel, per the head dimension):
  core c -> batch b = c // 4, heads {4g..4g+3} with g = c % 4.
  Each core computes Q,K,V for its 4 heads (512 qk cols + 256 v cols of the
  projection), runs causal attention for those heads, and multiplies by the
  matching 256 columns of Wo, producing a partial [2048, 1024] output (fp16).
  Host sums the 8 partials (4 per batch) in fp32.

Kernel structure (per core):
  - bf16 matmul operands (PE 1 cyc/row), fp32 PSUM accumulation.
  - Inputs are staged in DRAM in SBUF-matching tile-major order and DMA'd
    in consumption order: wqkT[ko0] + xT[qc0,ko0] first so the first
    projection matmul can issue ~3us in, then the rest of q-chunk 0's
    operands, then later q-chunks. Projection chunks (ko-outer) interleave
    with attention chunks.
  - Scores are computed TRANSPOSED (scoresT[k, q], head pairs packed in the
    PE via partition-base row tiling) so AV needs no transposes. Score
    blocks go into 2-bank PSUM tiles (two k-blocks per tile) so one
    ACTIVATE exps 1024 columns, halving ACT instruction overhead.
  - Causality: strictly-upper blocks skipped; diagonal straddlers compute
    only the valid columns; the 128x128 diagonal sub-block is exp'd
    unmasked then multiplied by a binary mask tile (from the mask input).
  - V carries a ones column (65 cols/head): AV's partition 64 accumulates
    the softmax denominator for free. Normalization = fast-approx
    reciprocal (sums >= 1), broadcast over partitions via gpsimd, one DVE
    multiply.
  - Output partials are written fp16 (verified: no accuracy impact at the
    harness tolerance), halving output DMA bytes.
"""

import os

import numpy as np

S = 2048
D = 1024
DH = 64
B = 2
NCORES = 8
HPC = 4  # heads per core
QKC = 2 * HPC * DH  # 512 q+k projection columns per core
VC = HPC * DH  # 256 v columns per core
P = 128
KO = D // P  # 8 contraction tiles
NQ = S // 512  # 4 q-chunks of 512
NSC = S // P  # 16 s-chunks of 128

COMPUTE_DT = os.environ.get("ATTN_COMPUTE_DT", "bf16")  # bf16 | f32r

_cache = {}


def _np_compute_dt():
    if COMPUTE_DT == "bf16":
        import ml_dtypes

        return ml_dtypes.bfloat16
    return np.float32


def _build():
    import concourse.bacc as bacc
    import concourse.mybir as mybir
    import concourse.tile as tile

    F32 = mybir.dt.float32
    F16 = mybir.dt.float16
    CDT = mybir.dt.bfloat16 if COMPUTE_DT == "bf16" else mybir.dt.float32r
    EXP = mybir.ActivationFunctionType.Exp

    nc = bacc.Bacc()
    # x is staged twice: per-(qc,ko) contiguous 128KB tiles for q-chunk 0
    # (fine-grained arrival tracking at startup) and p-major whole-slab
    # layout for q-chunks 1..3 (single transfer each, arrives early enough).
    x0_d = nc.dram_tensor("x0", [KO, P, 512], CDT, kind="ExternalInput")
    xr_d = nc.dram_tensor("xr", [NQ - 1, P, KO, 512], CDT, kind="ExternalInput")
    wqkT_d = nc.dram_tensor("wqkT", [KO, P, QKC], CDT, kind="ExternalInput")
    wvT_d = nc.dram_tensor("wvT", [KO, P, VC], CDT, kind="ExternalInput")
    woT_d = nc.dram_tensor("woT", [P, 2, D], CDT, kind="ExternalInput")
    maskT_d = nc.dram_tensor("maskT", [P, P], CDT, kind="ExternalInput")
    out_d = nc.dram_tensor("out", [S, D], F16, kind="ExternalOutput")

    with tile.TileContext(nc) as tc:
        with (
            tc.tile_pool(name="persist", bufs=1) as persist,
            tc.tile_pool(name="sb_small", bufs=3) as sb_small,
            tc.tile_pool(name="sb_exp", bufs=12) as sb_exp,
            tc.tile_pool(name="sb_out", bufs=3) as sb_out,
            tc.tile_pool(name="pp_big", bufs=2, space="PSUM") as pp_big,
            tc.tile_pool(name="pp_av", bufs=2, space="PSUM") as pp_av,
            tc.tile_pool(name="pp_o", bufs=2, space="PSUM") as pp_o,
        ):
            xT_sb = persist.tile([P, KO, S], CDT, tag="xT")
            wqkT_sb = persist.tile([P, KO, QKC], CDT, tag="wqkT")
            wvT_sb = persist.tile([P, KO, VC], CDT, tag="wvT")
            woT_sb = persist.tile([P, 2, D], CDT, tag="woT")
            maskT_sb = persist.tile([P, P], CDT, tag="maskT")
            qkT_sb = persist.tile([P, 4, S], CDT, tag="qkT")
            v_sb = persist.tile([P, NSC, HPC, DH + 1], CDT, tag="v")
            attn_sb = persist.tile([P, 2, S], CDT, tag="attn")

            # --- input DMAs in consumption order, per-ko granularity for
            # everything q-chunk 0's projections touch so the PE can track
            # arrival tile by tile. sync carries weights (first wqkT m-tile
            # split off so the first matmul can start on a 32KB transfer),
            # gpsimd carries x. xT for q-chunks 2/3 is emitted later on the
            # sync queue BEHIND output DMAs whose semaphores only clear
            # mid-kernel — engine queues run ahead of emission order, so
            # queue position behind a blocking wait is the only way to
            # actually defer those 2MB out of the congested startup window.
            nc.sync.dma_start(wqkT_sb[:, 0, 0:P], wqkT_d[0][:, 0:P])
            nc.gpsimd.dma_start(xT_sb[:, 0, 0:512], x0_d[0])
            nc.sync.dma_start(wqkT_sb[:, 0, P:QKC], wqkT_d[0][:, P:QKC])
            for ko in range(1, KO):
                nc.sync.dma_start(wqkT_sb[:, ko, :], wqkT_d[ko])
                nc.gpsimd.dma_start(xT_sb[:, ko, 0:512], x0_d[ko])
            for ko in range(KO):
                nc.sync.dma_start(wvT_sb[:, ko, :], wvT_d[ko])
            nc.sync.dma_start(maskT_sb[:], maskT_d[:])
            nc.gpsimd.dma_start(xT_sb[:, :, 512:1024], xr_d[0])
            nc.sync.dma_start(woT_sb[:], woT_d[:])

            ones_f32 = persist.tile([P, DH], F32, tag="ones_f32")
            nc.vector.memset(ones_f32[:], 1.0)
            nc.vector.tensor_copy(
                out=v_sb[:, :, :, DH],
                in_=ones_f32[:, 0 : NSC * HPC].rearrange("p (a b) -> p a b", a=NSC),
            )

            def emit_outproj_sc(sc):
                for en in range(2):
                    ps_o = pp_o.tile([P, 512], F32, tag="o")
                    for ct in range(2):
                        nc.tensor.matmul(
                            ps_o[:],
                            attn_sb[:, ct, sc * P : (sc + 1) * P],
                            woT_sb[:, ct, en * 512 : (en + 1) * 512],
                            start=(ct == 0),
                            stop=(ct == 1),
                        )
                    o_sb = sb_out.tile([P, 512], F16, tag="osb")
                    nc.vector.tensor_copy(out=o_sb[:], in_=ps_o[:])
                    eng = nc.sync if (sc + en) % 2 == 0 else nc.gpsimd
                    eng.dma_start(
                        out_d[sc * P : (sc + 1) * P, en * 512 : (en + 1) * 512],
                        o_sb[:],
                    )

            for qc in range(NQ):
                # --- qk projection chunk nn = qc, ko-outer over two 2-bank
                # tiles (4 half-bank chains), so PE tracks DMA arrival ---
                pjA = pp_big.tile([P, 1024], F32, tag="big", name="pjA")
                pjB = pp_big.tile([P, 1024], F32, tag="big", name="pjB")
                for ko in range(KO):
                    for mm in range(4):
                        slot = pjA if mm < 2 else pjB
                        nc.tensor.matmul(
                            slot[:, (mm % 2) * 512 : (mm % 2 + 1) * 512],
                            wqkT_sb[:, ko, mm * P : (mm + 1) * P],
                            xT_sb[:, ko, qc * 512 : (qc + 1) * 512],
                            start=(ko == 0),
                            stop=(ko == KO - 1),
                            skip_group_check=True,
                        )
                nc.vector.tensor_copy(
                    out=qkT_sb[:, 0:2, qc * 512 : (qc + 1) * 512],
                    in_=pjA.rearrange("p (a b) -> p a b", a=2),
                )
                nc.vector.tensor_copy(
                    out=qkT_sb[:, 2:4, qc * 512 : (qc + 1) * 512],
                    in_=pjB.rearrange("p (a b) -> p a b", a=2),
                )

                # --- v projection for s-chunks 4qc..4qc+3 (4 bank chains) ---
                pvA = pp_big.tile([P, 1024], F32, tag="big", name="pvA")
                pvB = pp_big.tile([P, 1024], F32, tag="big", name="pvB")
                for ko in range(KO):
                    for j in range(4):
                        slot = pvA if j < 2 else pvB
                        sc = 4 * qc + j
                        nc.tensor.matmul(
                            slot[:, (j % 2) * 512 : (j % 2) * 512 + VC],
                            xT_sb[:, ko, sc * P : (sc + 1) * P],
                            wvT_sb[:, ko, :],
                            start=(ko == 0),
                            stop=(ko == KO - 1),
                            skip_group_check=True,
                        )
                for half, slot in ((0, pvA), (1, pvB)):
                    nc.vector.tensor_copy(
                        out=v_sb[:, 4 * qc + 2 * half : 4 * qc + 2 * half + 2, :, 0:DH],
                        in_=slot.rearrange("p (a h d) -> p a h d", a=2, h=8)[:, :, 0:HPC, :],
                    )

                # --- attention for q-chunk qc ---
                nkb = 4 * qc + 4  # causal: k blocks 0 .. 4qc+3
                for h in range(HPC):
                    hp = (h % 2) * DH  # partition base within the m-tile
                    mq = h // 2  # Q m-tile; K m-tile = 2 + h//2
                    avs = []
                    for kb0 in range(0, nkb, 2):
                        ps2 = pp_big.tile([P, 1024], F32, tag="big", name="ps2")
                        exp2 = sb_exp.tile([P, 1024], CDT, tag="exp")
                        offs = []
                        for half in (0, 1):
                            kb = kb0 + half
                            m = kb - 4 * qc  # >= 0 on diagonal straddlers
                            off = max(0, m) * P
                            offs.append(off)
                            nc.tensor.matmul(
                                ps2[:, half * 512 + off : half * 512 + 512],
                                qkT_sb[hp : hp + DH, 2 + mq, kb * P : (kb + 1) * P],
                                qkT_sb[hp : hp + DH, mq, qc * 512 + off : (qc + 1) * 512],
                                start=True,
                                stop=True,
                                skip_group_check=True,
                            )
                        if offs[0] == 0 and offs[1] == 0:
                            # dense pair: one 1024-wide exp
                            nc.scalar.activation(exp2[:], ps2[:], EXP, scale=0.125)
                        else:
                            for half, off in enumerate(offs):
                                lo = half * 512 + off
                                nc.scalar.activation(
                                    exp2[:, lo : half * 512 + 512],
                                    ps2[:, lo : half * 512 + 512],
                                    EXP,
                                    scale=0.125,
                                )
                        for half, off in enumerate(offs):
                            kb = kb0 + half
                            if kb - 4 * qc >= 0:
                                lo = half * 512 + off
                                nc.vector.tensor_mul(
                                    out=exp2[:, lo : lo + P],
                                    in0=exp2[:, lo : lo + P],
                                    in1=maskT_sb[:],
                                )
                            avs.append((exp2, half * 512 + off, off, kb))
                    # deferred output projection (previous q chunk), one
                    # s-chunk per head, BETWEEN the scores run and the AV
                    # chain: ~1.7us of dependency-free PE work that covers
                    # ACT's catch-up on the last pair's exp, so the AV
                    # chain's final matmul never waits on the activation
                    # engine.
                    if qc > 0:
                        emit_outproj_sc(4 * (qc - 1) + h)
                        # xT for q-chunks 2/3: emitted on sync BEHIND the
                        # outproj output DMA above, whose semaphore only
                        # clears mid-kernel — a genuine deferral of these
                        # 2MB past the congested startup window. Needed at
                        # ~55us / ~75us respectively.
                        if qc == 1 and h in (0, 3):
                            qci = 2 if h == 0 else 3
                            nc.sync.dma_start(
                                xT_sb[:, :, qci * 512 : (qci + 1) * 512],
                                xr_d[qci - 1],
                            )
                    ps_av = pp_av.tile([DH + 1, 512], F32, tag="av")
                    for j, (exp2, lo, off, kb) in enumerate(avs):
                        nc.tensor.matmul(
                            ps_av[:, off:512],
                            v_sb[:, kb, h, :],
                            exp2[:, lo : (lo - off) + 512],
                            start=(j == 0),
                            stop=(j == len(avs) - 1),
                            skip_group_check=True,
                        )
                    # normalize: out = av * (1/sums) broadcast over partitions
                    sums_sb = sb_small.tile([1, 512], F32, tag="sums")
                    nc.vector.tensor_copy(out=sums_sb[:], in_=ps_av[DH : DH + 1, :])
                    recip_f = sb_small.tile([1, 512], F32, tag="recipf")
                    nc.vector.reciprocal_approx_fast(out=recip_f[:], in_=sums_sb[:])
                    bc_sb = sb_small.tile([DH, 512], F32, tag="bc")
                    nc.gpsimd.partition_broadcast(bc_sb[:], recip_f[:])
                    if qc == NQ - 1 and h == HPC - 1:
                        # final head: split the normalize multiply into
                        # 128-column pieces and pipeline the last four
                        # output projections behind them, shrinking the
                        # serial kernel tail.
                        for si in range(4):
                            nc.vector.tensor_mul(
                                out=attn_sb[
                                    hp : hp + DH,
                                    h // 2,
                                    (4 * qc + si) * P : (4 * qc + si + 1) * P,
                                ],
                                in0=ps_av[0:DH, si * P : (si + 1) * P],
                                in1=bc_sb[:, si * P : (si + 1) * P],
                            )
                            emit_outproj_sc(4 * qc + si)
                    else:
                        nc.vector.tensor_mul(
                            out=attn_sb[hp : hp + DH, h // 2, qc * 512 : (qc + 1) * 512],
                            in0=ps_av[0:DH, :],
                            in1=bc_sb[:],
                        )

    nc.compile()
    return nc


def _get_nc():
    if "nc" not in _cache:
        _cache["nc"] = _build()
    return _cache["nc"]


def _shard(x, mask, Wqkv, Wo):
    cdt = _np_compute_dt()
    in_maps = []
    # binary mask for the transposed 128x128 diagonal block:
    # valid (mask==0) -> 1.0, masked (-inf/large-negative) -> 0.0
    maskT = np.ascontiguousarray((mask[0, 0, :P, :P].T >= 0).astype(cdt))
    for c in range(NCORES):
        b = c // 4
        g = c % 4
        heads = [4 * g + i for i in range(HPC)]
        q_rows = np.concatenate([np.arange(h * DH, (h + 1) * DH) for h in heads])
        k_rows = D + q_rows
        v_rows = 2 * D + q_rows
        qk_rows = np.concatenate([q_rows, k_rows])
        xT = x[b].T.astype(cdt)  # [D, S]
        x4 = xT.reshape(KO, P, NQ, 512)
        # q-chunk 0 per-(ko) contiguous tiles: [KO, P, 512]
        x0 = np.ascontiguousarray(x4[:, :, 0, :])
        # q-chunks 1..3 p-major slabs: [NQ-1, P, KO, 512]
        xr = np.ascontiguousarray(x4[:, :, 1:, :].transpose(2, 1, 0, 3))
        # [KO, P, QKC]: W[ko, p, m] = Wqkv[qk_rows[m], ko*P+p]
        wqkT = np.ascontiguousarray(Wqkv[qk_rows, :].T.astype(cdt).reshape(KO, P, QKC))
        wvT = np.ascontiguousarray(Wqkv[v_rows, :].T.astype(cdt).reshape(KO, P, VC))
        # [P, 2, D]: woT[p, ct, e] = Wo[e, q_rows[ct*P+p]]
        woT = np.ascontiguousarray(
            Wo[:, q_rows].T.astype(cdt).reshape(2, P, D).transpose(1, 0, 2)
        )
        in_maps.append(
            {
                "x0": x0,
                "xr": xr,
                "wqkT": wqkT,
                "wvT": wvT,
                "woT": woT,
                "maskT": maskT,
            }
        )
    return in_maps


def kernel(x, mask, Wqkv, Wo, _trace=False):
    from concourse.bass_utils import run_bass_kernel_spmd

    x = np.asarray(x, dtype=np.float32)
    mask = np.asarray(mask, dtype=np.float32)
    Wqkv = np.asarray(Wqkv, dtype=np.float32)
    Wo = np.asarray(Wo, dtype=np.float32)

    nc = _get_nc()
    in_maps = _shard(x, mask, Wqkv, Wo)
    res = run_bass_kernel_spmd(nc, in_maps, core_ids=list(range(NCORES)), trace=_trace)
    _cache["last_result"] = res

    out = np.zeros((B, S, D), dtype=np.float32)
    for c in range(NCORES):
        out[c // 4] += res.results[c]["out"].astype(np.float32)
    return out


# revision 29
# speedup vs baseline: 1.0659x; 1.0236x over previous
"""Causal self-attention Trainium2 kernel (8-core SPMD).

Problem: x[2,2048,1024], causal mask, Wqkv[3072,1024], Wo[1024,1024], fp32.
  qkv = x @ Wqkv.T ; per-head causal softmax attention ; out = attn @ Wo.T

Sharding (data + tensor parallel, per the head dimension):
  core c -> batch b = c // 4, heads {4g..4g+3} with g = c % 4.
  Each core computes Q,K,V for its 4 heads (512 qk cols + 256 v cols of the
  projection), runs causal attention for those heads, and multiplies by the
  matching 256 columns of Wo, producing a partial [2048, 1024] output (fp16).
  Host sums the 8 partials (4 per batch) in fp32.

Kernel structure (per core):
  - bf16 matmul operands (PE 1 cyc/row), fp32 PSUM accumulation.
  - Inputs are staged in DRAM in SBUF-matching tile-major order and DMA'd
    in consumption order: wqkT[ko0] + xT[qc0,ko0] first so the first
    projection matmul can issue ~3us in, then the rest of q-chunk 0's
    operands, then later q-chunks. Projection chunks (ko-outer) interleave
    with attention chunks.
  - Scores are computed TRANSPOSED (scoresT[k, q], head pairs packed in the
    PE via partition-base row tiling) so AV needs no transposes. Score
    blocks go into 2-bank PSUM tiles (two k-blocks per tile) so one
    ACTIVATE exps 1024 columns, halving ACT instruction overhead.
  - Causality: strictly-upper blocks skipped; diagonal straddlers compute
    only the valid columns; the 128x128 diagonal sub-block is exp'd
    unmasked then multiplied by a binary mask tile (from the mask input).
  - V carries a ones column (65 cols/head): AV's partition 64 accumulates
    the softmax denominator for free. Normalization = fast-approx
    reciprocal (sums >= 1), broadcast over partitions via gpsimd, one DVE
    multiply.
  - Output partials are written fp16 (verified: no accuracy impact at the
    harness tolerance), halving output DMA bytes.
"""

import os

import numpy as np

S = 2048
D = 1024
DH = 64
B = 2
NCORES = 8
HPC = 4  # heads per core
QKC = 2 * HPC * DH  # 512 q+k projection columns per core
VC = HPC * DH  # 256 v columns per core
P = 128
KO = D // P  # 8 contraction tiles
NQ = S // 512  # 4 q-chunks of 512
NSC = S // P  # 16 s-chunks of 128

COMPUTE_DT = os.environ.get("ATTN_COMPUTE_DT", "bf16")  # bf16 | f32r

_cache = {}


def _np_compute_dt():
    if COMPUTE_DT == "bf16":
        import ml_dtypes

        return ml_dtypes.bfloat16
    return np.float32


def _build():
    import concourse.bacc as bacc
    import concourse.mybir as mybir
    import concourse.tile as tile

    F32 = mybir.dt.float32
    F16 = mybir.dt.float16
    CDT = mybir.dt.bfloat16 if COMPUTE_DT == "bf16" else mybir.dt.float32r
    EXP = mybir.ActivationFunctionType.Exp

    nc = bacc.Bacc()
    # x is staged twice: per-(qc,ko) contiguous 128KB tiles for q-chunk 0
    # (fine-grained arrival tracking at startup) and p-major whole-slab
    # layout for q-chunks 1..3 (single transfer each, arrives early enough).
    x0_d = nc.dram_tensor("x0", [KO, P, 512], CDT, kind="ExternalInput")
    xr_d = nc.dram_tensor("xr", [NQ - 1, P, KO, 512], CDT, kind="ExternalInput")
    wqkT_d = nc.dram_tensor("wqkT", [KO, P, QKC], CDT, kind="ExternalInput")
    wvT_d = nc.dram_tensor("wvT", [KO, P, VC], CDT, kind="ExternalInput")
    woT_d = nc.dram_tensor("woT", [P, 2, D], CDT, kind="ExternalInput")
    maskT_d = nc.dram_tensor("maskT", [P, P], CDT, kind="ExternalInput")
    out_d = nc.dram_tensor("out", [S, D], F16, kind="ExternalOutput")

    with tile.TileContext(nc) as tc:
        with (
            tc.tile_pool(name="persist", bufs=1) as persist,
            tc.tile_pool(name="sb_small", bufs=3) as sb_small,
            tc.tile_pool(name="sb_exp", bufs=12) as sb_exp,
            tc.tile_pool(name="sb_out", bufs=3) as sb_out,
            tc.tile_pool(name="pp_big", bufs=2, space="PSUM") as pp_big,
            tc.tile_pool(name="pp_av", bufs=2, space="PSUM") as pp_av,
            tc.tile_pool(name="pp_o", bufs=2, space="PSUM") as pp_o,
        ):
            xT_sb = persist.tile([P, KO, S], CDT, tag="xT")
            wqkT_sb = persist.tile([P, KO, QKC], CDT, tag="wqkT")
            wvT_sb = persist.tile([P, KO, VC], CDT, tag="wvT")
            woT_sb = persist.tile([P, 2, D], CDT, tag="woT")
            maskT_sb = persist.tile([P, P], CDT, tag="maskT")
            qkT_sb = persist.tile([P, 4, S], CDT, tag="qkT")
            v_sb = persist.tile([P, NSC, HPC, DH + 1], CDT, tag="v")
            attn_sb = persist.tile([P, 2, S], CDT, tag="attn")

            # --- input DMAs in consumption order, per-ko granularity for
            # everything q-chunk 0's projections touch so the PE can track
            # arrival tile by tile. sync carries weights (first wqkT m-tile
            # split off so the first matmul can start on a 32KB transfer),
            # gpsimd carries x. xT for q-chunks 2/3 is emitted later on the
            # sync queue BEHIND output DMAs whose semaphores only clear
            # mid-kernel — engine queues run ahead of emission order, so
            # queue position behind a blocking wait is the only way to
            # actually defer those 2MB out of the congested startup window.
            nc.sync.dma_start(wqkT_sb[:, 0, 0:P], wqkT_d[0][:, 0:P])
            nc.gpsimd.dma_start(xT_sb[:, 0, 0:512], x0_d[0])
            nc.sync.dma_start(wqkT_sb[:, 0, P:QKC], wqkT_d[0][:, P:QKC])
            for ko in range(1, KO):
                nc.sync.dma_start(wqkT_sb[:, ko, :], wqkT_d[ko])
                nc.gpsimd.dma_start(xT_sb[:, ko, 0:512], x0_d[ko])
            for ko in range(KO):
                nc.sync.dma_start(wvT_sb[:, ko, :], wvT_d[ko])
            nc.sync.dma_start(maskT_sb[:], maskT_d[:])
            nc.gpsimd.dma_start(xT_sb[:, :, 512:1024], xr_d[0])
            nc.gpsimd.dma_start(woT_sb[:], woT_d[:])

            ones_f32 = persist.tile([P, DH], F32, tag="ones_f32")
            nc.vector.memset(ones_f32[:], 1.0)
            nc.vector.tensor_copy(
                out=v_sb[:, :, :, DH],
                in_=ones_f32[:, 0 : NSC * HPC].rearrange("p (a b) -> p a b", a=NSC),
            )

            def emit_outproj_sc(sc):
                for en in range(2):
                    ps_o = pp_o.tile([P, 512], F32, tag="o")
                    for ct in range(2):
                        nc.tensor.matmul(
                            ps_o[:],
                            attn_sb[:, ct, sc * P : (sc + 1) * P],
                            woT_sb[:, ct, en * 512 : (en + 1) * 512],
                            start=(ct == 0),
                            stop=(ct == 1),
                        )
                    o_sb = sb_out.tile([P, 512], F16, tag="osb")
                    nc.vector.tensor_copy(out=o_sb[:], in_=ps_o[:])
                    eng = nc.sync if (sc + en) % 2 == 0 else nc.gpsimd
                    eng.dma_start(
                        out_d[sc * P : (sc + 1) * P, en * 512 : (en + 1) * 512],
                        o_sb[:],
                    )

            for qc in range(NQ):
                # --- qk projection chunk nn = qc, ko-outer over two 2-bank
                # tiles (4 half-bank chains), so PE tracks DMA arrival ---
                pjA = pp_big.tile([P, 1024], F32, tag="big", name="pjA")
                pjB = pp_big.tile([P, 1024], F32, tag="big", name="pjB")
                for ko in range(KO):
                    for mm in range(4):
                        slot = pjA if mm < 2 else pjB
                        nc.tensor.matmul(
                            slot[:, (mm % 2) * 512 : (mm % 2 + 1) * 512],
                            wqkT_sb[:, ko, mm * P : (mm + 1) * P],
                            xT_sb[:, ko, qc * 512 : (qc + 1) * 512],
                            start=(ko == 0),
                            stop=(ko == KO - 1),
                            skip_group_check=True,
                        )
                nc.scalar.copy(
                    out=qkT_sb[:, 0:2, qc * 512 : (qc + 1) * 512],
                    in_=pjA.rearrange("p (a b) -> p a b", a=2),
                )
                nc.scalar.copy(
                    out=qkT_sb[:, 2:4, qc * 512 : (qc + 1) * 512],
                    in_=pjB.rearrange("p (a b) -> p a b", a=2),
                )

                # --- v projection for s-chunks 4qc..4qc+3 (4 bank chains) ---
                pvA = pp_big.tile([P, 1024], F32, tag="big", name="pvA")
                pvB = pp_big.tile([P, 1024], F32, tag="big", name="pvB")
                for ko in range(KO):
                    for j in range(4):
                        slot = pvA if j < 2 else pvB
                        sc = 4 * qc + j
                        nc.tensor.matmul(
                            slot[:, (j % 2) * 512 : (j % 2) * 512 + VC],
                            xT_sb[:, ko, sc * P : (sc + 1) * P],
                            wvT_sb[:, ko, :],
                            start=(ko == 0),
                            stop=(ko == KO - 1),
                            skip_group_check=True,
                        )
                for half, slot in ((0, pvA), (1, pvB)):
                    nc.scalar.copy(
                        out=v_sb[:, 4 * qc + 2 * half : 4 * qc + 2 * half + 2, :, 0:DH],
                        in_=slot.rearrange("p (a h d) -> p a h d", a=2, h=8)[:, :, 0:HPC, :],
                    )

                # --- attention for q-chunk qc ---
                nkb = 4 * qc + 4  # causal: k blocks 0 .. 4qc+3
                for h in range(HPC):
                    hp = (h % 2) * DH  # partition base within the m-tile
                    mq = h // 2  # Q m-tile; K m-tile = 2 + h//2
                    avs = []
                    for kb0 in range(0, nkb, 2):
                        ps2 = pp_big.tile([P, 1024], F32, tag="big", name="ps2")
                        exp2 = sb_exp.tile([P, 1024], CDT, tag="exp")
                        offs = []
                        for half in (0, 1):
                            kb = kb0 + half
                            m = kb - 4 * qc  # >= 0 on diagonal straddlers
                            off = max(0, m) * P
                            offs.append(off)
                            nc.tensor.matmul(
                                ps2[:, half * 512 + off : half * 512 + 512],
                                qkT_sb[hp : hp + DH, 2 + mq, kb * P : (kb + 1) * P],
                                qkT_sb[hp : hp + DH, mq, qc * 512 + off : (qc + 1) * 512],
                                start=True,
                                stop=True,
                                skip_group_check=True,
                            )
                        if offs[0] == 0 and offs[1] == 0:
                            # dense pair: one 1024-wide exp
                            nc.scalar.activation(exp2[:], ps2[:], EXP, scale=0.125)
                        else:
                            for half, off in enumerate(offs):
                                lo = half * 512 + off
                                nc.scalar.activation(
                                    exp2[:, lo : half * 512 + 512],
                                    ps2[:, lo : half * 512 + 512],
                                    EXP,
                                    scale=0.125,
                                )
                        for half, off in enumerate(offs):
                            kb = kb0 + half
                            if kb - 4 * qc >= 0:
                                lo = half * 512 + off
                                nc.vector.tensor_mul(
                                    out=exp2[:, lo : lo + P],
                                    in0=exp2[:, lo : lo + P],
                                    in1=maskT_sb[:],
                                )
                            avs.append((exp2, half * 512 + off, off, kb))
                    # deferred output projection (previous q chunk), one
                    # s-chunk per head, BETWEEN the scores run and the AV
                    # chain: ~1.7us of dependency-free PE work that covers
                    # ACT's catch-up on the last pair's exp, so the AV
                    # chain's final matmul never waits on the activation
                    # engine.
                    if qc > 0:
                        emit_outproj_sc(4 * (qc - 1) + h)
                        # xT for q-chunks 2/3: emitted on sync BEHIND the
                        # outproj output DMA above, whose semaphore only
                        # clears mid-kernel — a genuine deferral of these
                        # 2MB past the congested startup window. Needed at
                        # ~55us / ~75us respectively.
                        if qc == 1 and h in (0, 3):
                            qci = 2 if h == 0 else 3
                            nc.sync.dma_start(
                                xT_sb[:, :, qci * 512 : (qci + 1) * 512],
                                xr_d[qci - 1],
                            )
                    ps_av = pp_av.tile([DH + 1, 512], F32, tag="av")
                    for j, (exp2, lo, off, kb) in enumerate(avs):
                        nc.tensor.matmul(
                            ps_av[:, off:512],
                            v_sb[:, kb, h, :],
                            exp2[:, lo : (lo - off) + 512],
                            start=(j == 0),
                            stop=(j == len(avs) - 1),
                            skip_group_check=True,
                        )
                    # normalize: out = av * (1/sums) broadcast over partitions
                    sums_sb = sb_small.tile([1, 512], F32, tag="sums")
                    nc.vector.tensor_copy(out=sums_sb[:], in_=ps_av[DH : DH + 1, :])
                    recip_f = sb_small.tile([1, 512], F32, tag="recipf")
                    nc.vector.reciprocal_approx_fast(out=recip_f[:], in_=sums_sb[:])
                    bc_sb = sb_small.tile([DH, 512], F32, tag="bc")
                    nc.gpsimd.partition_broadcast(bc_sb[:], recip_f[:])
                    if qc == NQ - 1 and h == HPC - 1:
                        # final head: split the normalize multiply into
                        # 128-column pieces and pipeline the last four
                        # output projections behind them, shrinking the
                        # serial kernel tail.
                        for si in range(4):
                            nc.vector.tensor_mul(
                                out=attn_sb[
                                    hp : hp + DH,
                                    h // 2,
                                    (4 * qc + si) * P : (4 * qc + si + 1) * P,
                                ],
                                in0=ps_av[0:DH, si * P : (si + 1) * P],
                                in1=bc_sb[:, si * P : (si + 1) * P],
                            )
                            emit_outproj_sc(4 * qc + si)
                    else:
                        nc.vector.tensor_mul(
                            out=attn_sb[hp : hp + DH, h // 2, qc * 512 : (qc + 1) * 512],
                            in0=ps_av[0:DH, :],
                            in1=bc_sb[:],
                        )

    nc.compile()
    return nc


def _get_nc():
    if "nc" not in _cache:
        _cache["nc"] = _build()
    return _cache["nc"]


def _shard(x, mask, Wqkv, Wo):
    cdt = _np_compute_dt()
    in_maps = []
    # binary mask for the transposed 128x128 diagonal block:
    # valid (mask==0) -> 1.0, masked (-inf/large-negative) -> 0.0
    maskT = np.ascontiguousarray((mask[0, 0, :P, :P].T >= 0).astype(cdt))
    for c in range(NCORES):
        b = c // 4
        g = c % 4
        heads = [4 * g + i for i in range(HPC)]
        q_rows = np.concatenate([np.arange(h * DH, (h + 1) * DH) for h in heads])
        k_rows = D + q_rows
        v_rows = 2 * D + q_rows
        qk_rows = np.concatenate([q_rows, k_rows])
        xT = x[b].T.astype(cdt)  # [D, S]
        x4 = xT.reshape(KO, P, NQ, 512)
        # q-chunk 0 per-(ko) contiguous tiles: [KO, P, 512]
        x0 = np.ascontiguousarray(x4[:, :, 0, :])
        # q-chunks 1..3 p-major slabs: [NQ-1, P, KO, 512]
        xr = np.ascontiguousarray(x4[:, :, 1:, :].transpose(2, 1, 0, 3))
        # [KO, P, QKC]: W[ko, p, m] = Wqkv[qk_rows[m], ko*P+p]
        wqkT = np.ascontiguousarray(Wqkv[qk_rows, :].T.astype(cdt).reshape(KO, P, QKC))
        wvT = np.ascontiguousarray(Wqkv[v_rows, :].T.astype(cdt).reshape(KO, P, VC))
        # [P, 2, D]: woT[p, ct, e] = Wo[e, q_rows[ct*P+p]]
        woT = np.ascontiguousarray(
            Wo[:, q_rows].T.astype(cdt).reshape(2, P, D).transpose(1, 0, 2)
        )
        in_maps.append(
            {
                "x0": x0,
                "xr": xr,
                "wqkT": wqkT,
                "wvT": wvT,
                "woT": woT,
                "maskT": maskT,
            }
        )
    return in_maps


def kernel(x, mask, Wqkv, Wo, _trace=False):
    from concourse.bass_utils import run_bass_kernel_spmd

    x = np.asarray(x, dtype=np.float32)
    mask = np.asarray(mask, dtype=np.float32)
    Wqkv = np.asarray(Wqkv, dtype=np.float32)
    Wo = np.asarray(Wo, dtype=np.float32)

    nc = _get_nc()
    in_maps = _shard(x, mask, Wqkv, Wo)
    res = run_bass_kernel_spmd(nc, in_maps, core_ids=list(range(NCORES)), trace=_trace)
    _cache["last_result"] = res

    out = np.zeros((B, S, D), dtype=np.float32)
    for c in range(NCORES):
        out[c // 4] += res.results[c]["out"].astype(np.float32)
    return out
